# revision 1
# baseline (speedup 1.0000x reference)
"""Causal self-attention (B=2, T=2048, C=1024, H=16) on 8 trn2 NeuronCores.

Sharding: data-parallel over B (2) x tensor-parallel over head groups (4),
so each of the 8 cores handles one batch element and 4 heads end-to-end:
QKV projection (its W_attn column slice), full-T causal attention for its
4 heads, and the partial output projection (its W_proj row slice). The
host sums the 4 per-batch partials and adds biases.

Device dataflow (matmuls in fp32r except P@V in bf16):
  x^T via PE transpose -> Q^T/K^T ([d, t] layout) and V ([t, d] layout)
  S^T[k, q] = K^T.T @ Q^T per head (causal block-skipped + trimmed)
  P = exp(S/8) on ScalarE (bf16), diagonal-block mask on VectorE
  y^T = (V|1).T @ P^T accumulated in PSUM (rowsum rides along)
  y^T *= 1/rowsum (reciprocal batched via partition-reshape DMAs)
  out^T = Wp_local.T @ y^T -> DRAM [1024, 2048] per core (host transposes/sums)
"""

import numpy as np

import concourse.bass as bass
import concourse.mybir as mybir
import concourse.tile as tile
from concourse import bacc
from concourse.bass_utils import run_bass_kernel_spmd

F32 = mybir.dt.float32
F32R = mybir.dt.float32r
BF16 = mybir.dt.bfloat16
AF = mybir.ActivationFunctionType

B, T, C, H = 2, 2048, 1024, 16
HD = C // H          # 64
NCORES = 8
CTILES = C // 128    # 8 contraction chunks
TT = T // 128        # 16 token tiles of 128
QG = T // 512        # 4 q-groups of 512


def build_nc(reps=1):
    nc = bacc.Bacc("TRN2", target_bir_lowering=False)

    x_d = nc.declare_dram_parameter("x_b", [T, C], F32R, isOutput=False)
    w_d = nc.declare_dram_parameter("w_l", [C, 768], F32R, isOutput=False)
    bqk_d = nc.declare_dram_parameter("b_qk", [4, 128], F32, isOutput=False)
    wp_d = nc.declare_dram_parameter("wp_l", [256, C], F32R, isOutput=False)
    out_d = nc.declare_dram_parameter("out_T", [C, T], F32, isOutput=True)

    with tile.TileContext(nc) as tc:
        for _ in range(reps):
            with tc.tile_pool(name="persist", bufs=1) as pp:
                _build_body(nc, tc, pp, x_d, w_d, bqk_d, wp_d, out_d)
    nc.compile()
    return nc


def _build_body(nc, tc, pp, x_d, w_d, bqk_d, wp_d, out_d):
    # ---- constants ----
    idf = pp.tile([128, 128], F32, tag="idf")
    nc.gpsimd.memset(idf, 0.0)
    nc.gpsimd.affine_select(out=idf, in_=idf, compare_op=mybir.AluOpType.not_equal,
                            fill=1.0, base=0, pattern=[[-1, 128]], channel_multiplier=1)
    ident = pp.tile([128, 128], F32R, tag="ident")
    nc.vector.tensor_copy(ident, idf)

    # causal mask for S^T diagonal blocks: keep where q(col) >= k(row)
    m0 = pp.tile([128, 128], BF16, tag="m0")
    nc.gpsimd.memset(m0, 1.0)
    nc.gpsimd.affine_select(out=m0, in_=m0, compare_op=mybir.AluOpType.is_ge,
                            fill=0.0, base=0, pattern=[[1, 128]], channel_multiplier=-1)

    onesrow_f = pp.tile([1, 128], F32, tag="onesrow_f")
    nc.vector.memset(onesrow_f, 1.0)
    onesrow = pp.tile([1, 128], F32R, tag="onesrow")
    nc.vector.tensor_copy(onesrow, onesrow_f)

    b_sb = pp.tile([128, 4], F32, tag="b_sb")
    nc.sync.dma_start(out=b_sb, in_=bqk_d.ap().rearrange("j p -> p j"))

    # ---- activations that span phase 1 -> 2 ----
    qk_pair = [pp.tile([128, T], F32R, tag=f"qkp{j}", name=f"qkp{j}") for j in range(4)]
    vt = [pp.tile([128, 260], BF16, tag=f"v{t}", name=f"v{t}") for t in range(TT)]
    for t in range(TT):
        nc.vector.memset(vt[t].rearrange("p (h c) -> p h c", c=65)[:, :, 64:65], 1.0)

    def QT(h):
        lo = 64 * (h % 2)
        return qk_pair[h // 2][lo:lo + 64, :]

    def KT(h):
        lo = 64 * (h % 2)
        return qk_pair[2 + h // 2][lo:lo + 64, :]

    # ================= phase 1: x^T, QKV =================
    with tc.tile_pool(name="ph1", bufs=1) as ph1, \
         tc.tile_pool(name="xn", bufs=1) as xn_pool, \
         tc.tile_pool(name="p1", bufs=2, space="PSUM") as p1:
        wt = [ph1.tile([128, 768], F32R, tag=f"w{ci}", name=f"w{ci}") for ci in range(CTILES)]
        xT = [ph1.tile([128, T], F32R, tag=f"xT{ci}", name=f"xT{ci}") for ci in range(CTILES)]

        def load_xns(tg):
            xns = []
            for tq in range(4):
                t = 4 * tg + tq
                xn = xn_pool.tile([128, C], F32R, tag=f"xn{tq}", name=f"xn{tq}")
                nc.sync.dma_start(out=xn[:, 0:512], in_=x_d[128 * t:128 * (t + 1), 0:512])
                nc.sync.dma_start(out=xn[:, 512:C], in_=x_d[128 * t:128 * (t + 1), 512:C])
                xns.append(xn)
            return xns

        nxt = load_xns(0)
        for ci in range(CTILES):
            nc.sync.dma_start(out=wt[ci], in_=w_d[128 * ci:128 * (ci + 1), :])

        for tg in range(QG):
            xns = nxt
            # transpose x block [512t x 1024c] -> xT chunks
            for ci in range(CTILES):
                pt = p1.tile([128, 512], F32R, tag="pt", bufs=3)
                for tq in range(4):
                    nc.tensor.transpose(pt[:, 128 * tq:128 * (tq + 1)],
                                        xns[tq][:, 128 * ci:128 * (ci + 1)], ident)
                nc.vector.tensor_copy(xT[ci][:, 512 * tg:512 * (tg + 1)], pt)
            if tg + 1 < QG:
                nxt = load_xns(tg + 1)
            # Q^T / K^T head-pair chunks (+bias), then odd-head relocation
            for jc in range(4):
                ps = p1.tile([128, 512], F32, tag="qk", bufs=3)
                for ci in range(CTILES):
                    nc.tensor.matmul(ps, wt[ci][:, 128 * jc:128 * (jc + 1)],
                                     xT[ci][:, 512 * tg:512 * (tg + 1)],
                                     start=(ci == 0), stop=(ci == CTILES - 1))
                nc.scalar.activation(qk_pair[jc][:, 512 * tg:512 * (tg + 1)], ps,
                                     AF.Identity, bias=b_sb[:, jc:jc + 1], scale=1.0)
            # V tiles [t, d] in bf16 with interleaved ones columns
            for tq in range(4):
                t = 4 * tg + tq
                pv = p1.tile([128, 256], F32, tag="v")
                for ci in range(CTILES):
                    nc.tensor.matmul(pv, xT[ci][:, 128 * t:128 * (t + 1)],
                                     wt[ci][:, 512:768],
                                     start=(ci == 0), stop=(ci == CTILES - 1))
                nc.vector.tensor_copy(
                    vt[t].rearrange("p (h c) -> p h c", c=65)[:, :, 0:64],
                    pv.rearrange("p (h c) -> p h c", c=64))

    # ================= phases 2+3 (shared scope so proj overlaps attention) ====
    with tc.tile_pool(name="p23", bufs=1) as p23, \
         tc.tile_pool(name="s", bufs=3, space="PSUM") as s_pool, \
         tc.tile_pool(name="y", bufs=1, space="PSUM") as y_pool, \
         tc.tile_pool(name="pb", bufs=5) as p_pool, \
         tc.tile_pool(name="stg", bufs=2) as st_pool, \
         tc.tile_pool(name="rr", bufs=2) as rr_pool, \
         tc.tile_pool(name="osb", bufs=8) as osb_pool:
        y_un = [p23.tile([128, T], F32R, tag=f"y{hp}", name=f"y{hp}") for hp in range(2)]
        # packed softmax denominators: slot s=hp*4+qi; head-A at cols [4s:4s+4],
        # head-B at cols [32+4s:32+4s+4]; each [1,512] psum row spread over partitions
        sq = p23.tile([128, 64], F32R, tag="sq")
        ri = p23.tile([128, 64], F32R, tag="ri")
        wp = [p23.tile([128, C], F32R, tag=f"wp{k}", name=f"wp{k}") for k in range(2)]
        for k in range(2):
            nc.sync.dma_start(out=wp[k], in_=wp_d[128 * k:128 * (k + 1), :])

        # ---- phase 2: attention ----
        for hp in range(2):
            hA, hB = 2 * hp, 2 * hp + 1
            for qi in range(QG):
                psA = y_pool.tile([128, 512], F32, tag="yA")
                psB = y_pool.tile([128, 512], F32, tag="yB")
                nkt = 4 * qi + 4
                for ki in range(nkt):
                    r = ki - 4 * qi
                    soff = 0 if r < 1 else (128 * r if r < 3 else 256)
                    sAB = s_pool.tile([128, 1024], F32, tag="s")
                    for half, h in ((0, hA), (1, hB)):
                        nc.tensor.matmul(
                            sAB[:, 512 * half + soff:512 * half + 512],
                            KT(h)[:, 128 * ki:128 * (ki + 1)],
                            QT(h)[:, 512 * qi + soff:512 * (qi + 1)],
                            start=True, stop=True)
                    pAB = p_pool.tile([128, 1024], BF16, tag="p")
                    if r >= 1:
                        we = 128 * r
                        nc.scalar.activation(
                            pAB.rearrange("p (h q) -> p h q", h=2)[:, :, we:512],
                            sAB.rearrange("p (h q) -> p h q", h=2)[:, :, we:512],
                            AF.Exp, scale=1.0 / np.sqrt(HD))
                    else:
                        nc.scalar.activation(pAB, sAB, AF.Exp, scale=1.0 / np.sqrt(HD))
                    if r >= 0:
                        for half in range(2):
                            base = 512 * half + 128 * r
                            nc.vector.tensor_mul(pAB[:, base:base + 128],
                                                 pAB[:, base:base + 128], m0)
                    woff = 0 if r < 0 else 128 * r
                    st, sp = (ki == 0), (ki == nkt - 1)
                    nc.tensor.matmul(psA[0:65, woff:512],
                                     vt[ki][:, 65 * hA:65 * hA + 65],
                                     pAB[:, woff:512],
                                     start=st, stop=sp)
                    nc.tensor.matmul(psB[0:65, woff:512],
                                     vt[ki][:, 65 * hB:65 * hB + 65],
                                     pAB[:, 512 + woff:1024],
                                     start=st, stop=sp)
                sl = hp * QG + qi
                stA = st_pool.tile([128, 512], F32R, tag="st", name="stA")
                stB = st_pool.tile([128, 512], F32R, tag="st", name="stB")
                nc.vector.tensor_copy(stA[64:65, :], psA[64:65, :])
                nc.vector.tensor_copy(stB[0:65, :], psB[0:65, :])
                nc.sync.dma_start(out=sq[:, 4 * sl:4 * sl + 4], in_=stA[64:65, :])
                nc.sync.dma_start(out=sq[:, 32 + 4 * sl:32 + 4 * sl + 4], in_=stB[64:65, :])
                nc.vector.tensor_copy(y_un[hp][0:64, 512 * qi:512 * (qi + 1)], psA[0:64, :])
                nc.sync.dma_start(out=y_un[hp][64:128, 512 * qi:512 * (qi + 1)],
                                  in_=stB[0:64, :])
                with nc.allow_low_precision(reason="softmax denominator reciprocal"):
                    nc.vector.reciprocal(ri[:, 4 * sl:4 * sl + 4], sq[:, 4 * sl:4 * sl + 4])
                    nc.vector.reciprocal(ri[:, 32 + 4 * sl:32 + 4 * sl + 4],
                                         sq[:, 32 + 4 * sl:32 + 4 * sl + 4])
                rrA = rr_pool.tile([1, 512], F32R, tag="rrA")
                rrB = rr_pool.tile([1, 512], F32R, tag="rrB")
                nc.sync.dma_start(out=rrA, in_=ri[:, 4 * sl:4 * sl + 4])
                nc.sync.dma_start(out=rrB, in_=ri[:, 32 + 4 * sl:32 + 4 * sl + 4])
                rbA = rr_pool.tile([128, 512], F32R, tag="rbA")
                rbB = rr_pool.tile([128, 512], F32R, tag="rbB")
                nc.gpsimd.partition_broadcast(rbA, rrA, channels=128)
                nc.gpsimd.partition_broadcast(rbB, rrB, channels=128)
                nc.vector.tensor_mul(y_un[hp][0:64, 512 * qi:512 * (qi + 1)],
                                     y_un[hp][0:64, 512 * qi:512 * (qi + 1)],
                                     rbA[0:64, :])
                nc.vector.tensor_mul(y_un[hp][64:128, 512 * qi:512 * (qi + 1)],
                                     y_un[hp][64:128, 512 * qi:512 * (qi + 1)],
                                     rbB[64:128, :])

        # ---- phase 3: output projection (tg-outer so it can chase phase 2) ----
        for tg in range(QG):
            for co in range(CTILES):
                pr = s_pool.tile([128, 512], F32, tag="s", name="pr")
                for k in range(2):
                    nc.tensor.matmul(pr, wp[k][:, 128 * co:128 * (co + 1)],
                                     y_un[k][:, 512 * tg:512 * (tg + 1)],
                                     start=(k == 0), stop=(k == 1))
                osb = osb_pool.tile([128, 512], F32, tag="osb")
                if co % 2 == 0:
                    nc.scalar.copy(osb, pr)
                else:
                    nc.vector.tensor_copy(osb, pr)
                nc.sync.dma_start(
                    out=out_d[128 * co:128 * (co + 1), 512 * tg:512 * (tg + 1)],
                    in_=osb)


_NC = None


def _get_nc():
    global _NC
    if _NC is None:
        _NC = build_nc()
    return _NC


def kernel(x, W_attn, b_attn, W_proj, b_proj, _trace=False):
    x = np.asarray(x, dtype=np.float32)
    W_attn = np.asarray(W_attn, dtype=np.float32)
    b_attn = np.asarray(b_attn, dtype=np.float32)
    W_proj = np.asarray(W_proj, dtype=np.float32)
    b_proj = np.asarray(b_proj, dtype=np.float32)

    in_maps = []
    for core in range(NCORES):
        b, hg = divmod(core, 4)
        qs = [W_attn[:, 64 * (4 * hg + h):64 * (4 * hg + h + 1)] for h in range(4)]
        ks = [W_attn[:, C + 64 * (4 * hg + h):C + 64 * (4 * hg + h + 1)] for h in range(4)]
        vs = [W_attn[:, 2 * C + 64 * (4 * hg + h):2 * C + 64 * (4 * hg + h + 1)] for h in range(4)]
        w_l = np.concatenate(qs + ks + vs, axis=1)
        bq = [b_attn[64 * (4 * hg + h):64 * (4 * hg + h + 1)] for h in range(4)]
        bk = [b_attn[C + 64 * (4 * hg + h):C + 64 * (4 * hg + h + 1)] for h in range(4)]
        b_qk = np.stack([np.concatenate(bq[0:2]), np.concatenate(bq[2:4]),
                         np.concatenate(bk[0:2]), np.concatenate(bk[2:4])])
        wp_l = np.concatenate(
            [W_proj[64 * (4 * hg + h):64 * (4 * hg + h + 1), :] for h in range(4)], axis=0)
        in_maps.append({
            "x_b": np.ascontiguousarray(x[b], dtype=np.float32),
            "w_l": np.ascontiguousarray(w_l, dtype=np.float32),
            "b_qk": np.ascontiguousarray(b_qk, dtype=np.float32),
            "wp_l": np.ascontiguousarray(wp_l, dtype=np.float32),
        })

    nc = _get_nc()
    kwargs = {}
    if _trace:
        kwargs = dict(trace=True, trace_cores=[0])
    res = run_bass_kernel_spmd(nc, in_maps, core_ids=list(range(NCORES)), **kwargs)

    # V-bias folds into the output bias because softmax rows sum to 1.
    bias_total = b_proj + b_attn[2 * C:3 * C] @ W_proj
    out = np.empty((B, T, C), dtype=np.float32)
    for b in range(B):
        acc = res.results[4 * b]["out_T"].astype(np.float32).copy()
        for hg in range(1, 4):
            acc += res.results[4 * b + hg]["out_T"]
        out[b] = acc.T + bias_total[None, :]
    if _trace:
        return out, res
    return out



# revision 3
# speedup vs baseline: 1.1345x; 1.1345x over previous
"""Causal self-attention (B=2, T=2048, C=1024, H=16) on 8 trn2 NeuronCores.

Sharding: data-parallel over B (2) x tensor-parallel over head groups (4),
so each of the 8 cores handles one batch element and 4 heads end-to-end:
QKV projection (its W_attn column slice), full-T causal attention for its
4 heads, and the partial output projection (its W_proj row slice). The
host sums the 4 per-batch partials and adds biases.

Device dataflow (all matmuls bf16; host pre-transposes and pre-casts):
  x^T uploaded directly as bf16 [C, T] (no on-device transpose)
  QKV(tg): Q^T/K^T ([d, t] bf16) and V ([t, d] bf16) per 512-token group
  S^T[k, q] = K^T.T @ Q^T per head (causal block-skipped + trimmed)
  P = exp(S/8) on ScalarE (bf16), diagonal-block mask on VectorE
  y^T = (V|1).T @ P^T accumulated in PSUM (rowsum rides along)
  y^T *= 1/rowsum (reciprocal batched via partition-reshape DMAs)
  out^T = Wp_local.T @ y^T -> DRAM bf16 [1024, 2048] per core

Software pipeline (qi-outer): QKV(tg+1) and proj(tg-1) matmuls are
interleaved as filler into attention(qi=tg)'s S->exp->PV loop so the PE
never starves while ScalarE runs exp.
"""

import numpy as np
import ml_dtypes

import concourse.bass as bass
import concourse.mybir as mybir
import concourse.tile as tile
from concourse import bacc
from concourse.bass_utils import run_bass_kernel_spmd

F32 = mybir.dt.float32
F32R = mybir.dt.float32r
BF16 = mybir.dt.bfloat16
AF = mybir.ActivationFunctionType

B, T, C, H = 2, 2048, 1024, 16
HD = C // H          # 64
NCORES = 8
CTILES = C // 128    # 8 contraction chunks
TT = T // 128        # 16 token tiles of 128
QG = T // 512        # 4 q-groups of 512
SCL = 1.0 / float(np.sqrt(HD))


def build_nc():
    nc = bacc.Bacc("TRN2", target_bir_lowering=False)

    xT_d = nc.declare_dram_parameter("xT_b", [C, T], BF16, isOutput=False)
    w_d = nc.declare_dram_parameter("w_l", [C, 768], BF16, isOutput=False)
    bqk_d = nc.declare_dram_parameter("b_qk", [4, 128], F32, isOutput=False)
    wp_d = nc.declare_dram_parameter("wp_l", [256, C], BF16, isOutput=False)
    out_d = nc.declare_dram_parameter("out_T", [C, T], BF16, isOutput=True)

    with tile.TileContext(nc) as tc:
        with tc.tile_pool(name="persist", bufs=1) as pp:
            _build_body(nc, tc, pp, xT_d, w_d, bqk_d, wp_d, out_d)
    nc.compile()
    return nc


def _build_body(nc, tc, pp, xT_d, w_d, bqk_d, wp_d, out_d):
    # ---- constants ----
    # causal mask for S^T diagonal blocks: keep where q(col) >= k(row)
    m0 = pp.tile([128, 128], BF16, tag="m0")
    nc.gpsimd.memset(m0, 1.0)
    nc.gpsimd.affine_select(out=m0, in_=m0, compare_op=mybir.AluOpType.is_ge,
                            fill=0.0, base=0, pattern=[[1, 128]], channel_multiplier=-1)

    b_sb = pp.tile([128, 4], F32, tag="b_sb")
    nc.sync.dma_start(out=b_sb, in_=bqk_d.ap().rearrange("j p -> p j"))

    # ---- persistent tiles ----
    wt = [pp.tile([128, 768], BF16, tag=f"w{ci}", name=f"w{ci}") for ci in range(CTILES)]
    wp = [pp.tile([128, C], BF16, tag=f"wp{k}", name=f"wp{k}") for k in range(2)]
    xT = [pp.tile([128, T], BF16, tag=f"xT{ci}", name=f"xT{ci}") for ci in range(CTILES)]
    qk_pair = [pp.tile([128, T], BF16, tag=f"qkp{j}", name=f"qkp{j}") for j in range(4)]
    vt = [pp.tile([128, 260], BF16, tag=f"v{t}", name=f"v{t}") for t in range(TT)]
    y_un = [pp.tile([128, T], BF16, tag=f"y{hp}", name=f"y{hp}") for hp in range(2)]
    # packed softmax denominators: slot sl=hp*4+qi; head-A at cols [4sl:4sl+4],
    # head-B at cols [32+4sl:32+4sl+4]; each [1,512] row spread over partitions
    sq = pp.tile([128, 64], BF16, tag="sq")
    ri = pp.tile([128, 64], BF16, tag="ri")

    for t in range(TT):
        nc.vector.memset(vt[t].rearrange("p (h c) -> p h c", c=65)[:, :, 64:65], 1.0)

    def QT(h):
        lo = 64 * (h % 2)
        return qk_pair[h // 2][lo:lo + 64, :]

    def KT(h):
        lo = 64 * (h % 2)
        return qk_pair[2 + h // 2][lo:lo + 64, :]

    def load_xT(tg):
        for ci in range(CTILES):
            nc.sync.dma_start(out=xT[ci][:, 512 * tg:512 * (tg + 1)],
                              in_=xT_d[128 * ci:128 * (ci + 1), 512 * tg:512 * (tg + 1)])

    # startup DMAs: weights + first x slice interleaved so QKV(0) starts early
    for ci in range(CTILES):
        nc.sync.dma_start(out=wt[ci], in_=w_d[128 * ci:128 * (ci + 1), :])
        nc.sync.dma_start(out=xT[ci][:, 0:512], in_=xT_d[128 * ci:128 * (ci + 1), 0:512])
    for k in range(2):
        nc.sync.dma_start(out=wp[k], in_=wp_d[128 * k:128 * (k + 1), :])

    with tc.tile_pool(name="qv", bufs=2, space="PSUM") as qv_pool, \
         tc.tile_pool(name="s", bufs=2, space="PSUM") as s_pool, \
         tc.tile_pool(name="y", bufs=1, space="PSUM") as y_pool, \
         tc.tile_pool(name="pb", bufs=5) as p_pool, \
         tc.tile_pool(name="stg", bufs=2) as st_pool, \
         tc.tile_pool(name="rr", bufs=2) as rr_pool, \
         tc.tile_pool(name="osb", bufs=8) as osb_pool:

        state = {}

        def qkv_steps(tg):
            """One step per contraction-chunk matmul; Q/K blocks then V tiles."""
            steps = []
            for jc in range(4):
                for ci in range(CTILES):
                    def step(jc=jc, ci=ci, tg=tg):
                        if ci == 0:
                            state[("qk", jc)] = qv_pool.tile([128, 512], F32, tag="qv", name=f"qk{jc}")
                        ps = state[("qk", jc)]
                        nc.tensor.matmul(ps, wt[ci][:, 128 * jc:128 * (jc + 1)],
                                         xT[ci][:, 512 * tg:512 * (tg + 1)],
                                         start=(ci == 0), stop=(ci == CTILES - 1))
                        if ci == CTILES - 1:
                            nc.scalar.activation(qk_pair[jc][:, 512 * tg:512 * (tg + 1)],
                                                 ps, AF.Identity,
                                                 bias=b_sb[:, jc:jc + 1], scale=1.0)
                    steps.append(step)
            for tq in range(4):
                t = 4 * tg + tq
                for ci in range(CTILES):
                    def step(t=t, ci=ci):
                        if ci == 0:
                            state[("v", t)] = qv_pool.tile([128, 512], F32, tag="qv", name=f"pv{t}")
                        pv = state[("v", t)]
                        nc.tensor.matmul(pv[:, 0:256], xT[ci][:, 128 * t:128 * (t + 1)],
                                         wt[ci][:, 512:768],
                                         start=(ci == 0), stop=(ci == CTILES - 1))
                        if ci == CTILES - 1:
                            nc.vector.tensor_copy(
                                vt[t].rearrange("p (h c) -> p h c", c=65)[:, :, 0:64],
                                pv[:, 0:256].rearrange("p (h c) -> p h c", c=64))
                    steps.append(step)
            return steps

        def proj_steps(tg):
            steps = []
            for co in range(CTILES):
                def mm(co=co, tg=tg):
                    pr = qv_pool.tile([128, 512], F32, tag="qv", name=f"pr{co}")
                    state[("pr", co)] = pr
                    nc.tensor.matmul(pr, wp[0][:, 128 * co:128 * (co + 1)],
                                     y_un[0][:, 512 * tg:512 * (tg + 1)],
                                     start=True, stop=False)
                    nc.tensor.matmul(pr, wp[1][:, 128 * co:128 * (co + 1)],
                                     y_un[1][:, 512 * tg:512 * (tg + 1)],
                                     start=False, stop=True)

                def wr(co=co, tg=tg):
                    pr = state[("pr", co)]
                    osb = osb_pool.tile([128, 512], BF16, tag="osb")
                    if co % 2 == 0:
                        nc.scalar.copy(osb, pr)
                    else:
                        nc.vector.tensor_copy(osb, pr)
                    nc.sync.dma_start(
                        out=out_d[128 * co:128 * (co + 1), 512 * tg:512 * (tg + 1)],
                        in_=osb)
                steps.append(mm)
                steps.append(wr)
            return steps

        def emit_pv(psA, psB, ki, pAB, hA, hB, woff, st, sp):
            nc.tensor.matmul(psA[0:65, woff:512],
                             vt[ki][:, 65 * hA:65 * hA + 65],
                             pAB[:, woff:512], start=st, stop=sp)
            nc.tensor.matmul(psB[0:65, woff:512],
                             vt[ki][:, 65 * hB:65 * hB + 65],
                             pAB[:, 512 + woff:1024], start=st, stop=sp)

        def epilogue(qi, hp, psA, psB):
            sl = hp * QG + qi
            stA = st_pool.tile([128, 512], BF16, tag="st", name="stA")
            stB = st_pool.tile([128, 512], BF16, tag="st", name="stB")
            nc.vector.tensor_copy(stA[64:65, :], psA[64:65, :])
            nc.vector.tensor_copy(stB[0:65, :], psB[0:65, :])
            nc.sync.dma_start(out=sq[:, 4 * sl:4 * sl + 4], in_=stA[64:65, :])
            nc.sync.dma_start(out=sq[:, 32 + 4 * sl:32 + 4 * sl + 4], in_=stB[64:65, :])
            nc.vector.tensor_copy(y_un[hp][0:64, 512 * qi:512 * (qi + 1)], psA[0:64, :])
            nc.sync.dma_start(out=y_un[hp][64:128, 512 * qi:512 * (qi + 1)],
                              in_=stB[0:64, :])
            with nc.allow_low_precision(reason="softmax denominator reciprocal"):
                nc.vector.reciprocal(ri[:, 4 * sl:4 * sl + 4], sq[:, 4 * sl:4 * sl + 4])
                nc.vector.reciprocal(ri[:, 32 + 4 * sl:32 + 4 * sl + 4],
                                     sq[:, 32 + 4 * sl:32 + 4 * sl + 4])
            rrA = rr_pool.tile([1, 512], BF16, tag="rrA")
            rrB = rr_pool.tile([1, 512], BF16, tag="rrB")
            nc.sync.dma_start(out=rrA, in_=ri[:, 4 * sl:4 * sl + 4])
            nc.sync.dma_start(out=rrB, in_=ri[:, 32 + 4 * sl:32 + 4 * sl + 4])
            rbA = rr_pool.tile([128, 512], BF16, tag="rbA")
            rbB = rr_pool.tile([128, 512], BF16, tag="rbB")
            nc.gpsimd.partition_broadcast(rbA, rrA, channels=128)
            nc.gpsimd.partition_broadcast(rbB, rrB, channels=128)
            with nc.allow_low_precision(reason="softmax normalize in bf16"):
                nc.vector.tensor_mul(y_un[hp][0:64, 512 * qi:512 * (qi + 1)],
                                     y_un[hp][0:64, 512 * qi:512 * (qi + 1)],
                                     rbA[0:64, :])
                nc.vector.tensor_mul(y_un[hp][64:128, 512 * qi:512 * (qi + 1)],
                                     y_un[hp][64:128, 512 * qi:512 * (qi + 1)],
                                     rbB[64:128, :])

        def attention(qi, filler):
            nkt = 4 * qi + 4
            slots = 2 * nkt
            nf = len(filler)
            prog = {"done": 0, "slot": 0}

            def pop_for_slot():
                prog["slot"] += 1
                target = (nf * prog["slot"]) // slots
                while prog["done"] < target:
                    filler[prog["done"]]()
                    prog["done"] += 1

            for hp in range(2):
                hA, hB = 2 * hp, 2 * hp + 1
                psA = y_pool.tile([128, 512], F32, tag="yA")
                psB = y_pool.tile([128, 512], F32, tag="yB")
                pend = None
                for ki in range(nkt):
                    r = ki - 4 * qi
                    soff = 0 if r < 1 else 128 * r
                    sAB = s_pool.tile([128, 1024], F32, tag="s")
                    for half, h in ((0, hA), (1, hB)):
                        nc.tensor.matmul(
                            sAB[:, 512 * half + soff:512 * half + 512],
                            KT(h)[:, 128 * ki:128 * (ki + 1)],
                            QT(h)[:, 512 * qi + soff:512 * (qi + 1)],
                            start=True, stop=True)
                    pAB = p_pool.tile([128, 1024], BF16, tag="p")
                    if r >= 1:
                        we = 128 * r
                        nc.scalar.activation(
                            pAB.rearrange("p (h q) -> p h q", h=2)[:, :, we:512],
                            sAB.rearrange("p (h q) -> p h q", h=2)[:, :, we:512],
                            AF.Exp, scale=SCL)
                    else:
                        nc.scalar.activation(pAB, sAB, AF.Exp, scale=SCL)
                    if r >= 0:
                        for half in range(2):
                            base = 512 * half + 128 * r
                            nc.vector.tensor_mul(pAB[:, base:base + 128],
                                                 pAB[:, base:base + 128], m0)
                    pop_for_slot()
                    if pend is not None:
                        emit_pv(*pend)
                    pend = (psA, psB, ki, pAB, hA, hB, 0 if r < 0 else 128 * r,
                            ki == 0, ki == nkt - 1)
                emit_pv(*pend)
                epilogue(qi, hp, psA, psB)

        # ---- pipelined main sequence ----
        for step in qkv_steps(0):
            step()
        load_xT(1)
        for qi in range(QG):
            filler = []
            if qi + 1 < QG:
                filler += qkv_steps(qi + 1)
            if qi >= 1:
                filler += proj_steps(qi - 1)
            attention(qi, filler)
            if qi + 2 < QG:
                load_xT(qi + 2)
        for step in proj_steps(QG - 1):
            step()


_NC = None


def _get_nc():
    global _NC
    if _NC is None:
        _NC = build_nc()
    return _NC


def kernel(x, W_attn, b_attn, W_proj, b_proj, _trace=False):
    x = np.asarray(x, dtype=np.float32)
    W_attn = np.asarray(W_attn, dtype=np.float32)
    b_attn = np.asarray(b_attn, dtype=np.float32)
    W_proj = np.asarray(W_proj, dtype=np.float32)
    b_proj = np.asarray(b_proj, dtype=np.float32)

    BF = ml_dtypes.bfloat16
    xTs = [np.ascontiguousarray(x[b].T.astype(BF)) for b in range(B)]
    in_maps = []
    for core in range(NCORES):
        b, hg = divmod(core, 4)
        qs = [W_attn[:, 64 * (4 * hg + h):64 * (4 * hg + h + 1)] for h in range(4)]
        ks = [W_attn[:, C + 64 * (4 * hg + h):C + 64 * (4 * hg + h + 1)] for h in range(4)]
        vs = [W_attn[:, 2 * C + 64 * (4 * hg + h):2 * C + 64 * (4 * hg + h + 1)] for h in range(4)]
        w_l = np.concatenate(qs + ks + vs, axis=1).astype(BF)
        bq = [b_attn[64 * (4 * hg + h):64 * (4 * hg + h + 1)] for h in range(4)]
        bk = [b_attn[C + 64 * (4 * hg + h):C + 64 * (4 * hg + h + 1)] for h in range(4)]
        b_qk = np.stack([np.concatenate(bq[0:2]), np.concatenate(bq[2:4]),
                         np.concatenate(bk[0:2]), np.concatenate(bk[2:4])])
        wp_l = np.concatenate(
            [W_proj[64 * (4 * hg + h):64 * (4 * hg + h + 1), :] for h in range(4)],
            axis=0).astype(BF)
        in_maps.append({
            "xT_b": xTs[b],
            "w_l": np.ascontiguousarray(w_l),
            "b_qk": np.ascontiguousarray(b_qk.astype(np.float32)),
            "wp_l": np.ascontiguousarray(wp_l),
        })

    nc = _get_nc()
    kwargs = {}
    if _trace:
        kwargs = dict(trace=True, trace_cores=[0])
    res = run_bass_kernel_spmd(nc, in_maps, core_ids=list(range(NCORES)), **kwargs)

    # V-bias folds into the output bias because softmax rows sum to 1.
    bias_total = b_proj + b_attn[2 * C:3 * C] @ W_proj
    out = np.empty((B, T, C), dtype=np.float32)
    for b in range(B):
        acc = res.results[4 * b]["out_T"].astype(np.float32)
        for hg in range(1, 4):
            acc = acc + res.results[4 * b + hg]["out_T"].astype(np.float32)
        out[b] = acc.T + bias_total[None, :]
    if _trace:
        return out, res
    return out
